# revision 1
# baseline (speedup 1.0000x reference)
"""Trainium2 Bass kernel for nn_EuESN_maml: assemble the 3N x 3N wave-equation
transition matrix A (N = 48*48) from c/dkx/dky fields.

A is all zeros except ~11 diagonals, so the kernel is a DMA memset of the
output plus strided diagonal scatter writes.

Sharding (SPMD, 8 cores): block-row index partitioned. Core k owns rows
[288k, 288k+288) of each of the three N-row block rows of A -> a [864, 6912]
shard per core. Each 288-row sub-band is written column-rotated by its first
global row index so diagonal positions are identical on every core (single
SPMD program); the host un-rotates with two slice copies while gathering.

Engine plan per core:
  vector: memset the zero tile (4 quarters), then the 11 diag value vectors
  sync  (HWDGE ring 0): 8 large contiguous ~3MB DMAs zero-fill the shard at
         ~355 GB/s (the ~358 GB/s per-core HBM cap); the first chunk goes in
         four quarter-DMAs chasing the quarter-memsets
  scalar(HWDGE ring 1): strided diagonal scatter DMAs, issued per sub-band as
         soon as the covering zero chunks have landed (overlaps the fill);
         sub-band 2 is split so only the last chunk's rows wait for fill end
"""

import math
import sys

import numpy as np

sys.path.insert(0, "/opt/trn_rl_repo")

import concourse.bass as bass
import concourse.mybir as mybir
from concourse.bass_utils import run_bass_kernel_spmd

# ---- problem constants (hardcoded from the nn_EuESN_maml spec) ----
n = 48
N = n * n            # 2304
M3 = 3 * N           # 6912 (output is M3 x M3)
NCORES = 8
B = N // NCORES      # 288 rows per sub-band
ROWS = 3 * B         # 864 rows per core shard
DT, CN, KP = 1.0, 0.1, 1e-4
MI = 1.0 / (1.0 / DT - KP / 2.0)          # 1/diagM (diagM is constant)
K0 = (1.0 / DT + KP / 2.0) * MI           # A00 diagonal value (constant)
DXC = (DT / CN) * math.sqrt(2.0)          # dx = DXC * max(c)

# zero-fill: NCHUNK contiguous chunks of [128, ZW] elements each
ZW = 5832
NCHUNK = (ROWS * M3) // (128 * ZW)        # 8
CHUNK_ROWS = 128 * ZW // M3               # 108 shard rows per chunk

# packed per-core input: [c.flat (N)] + 8 vectors of length B
PK = N + 8 * B

# number of chained vector-engine ops (vchain semaphore target)
NVOPS = 28

NSLOTS = 11

# scatter DMAs: (sub_band, col0, kind, slot/base)
# kind "s": one diagonal from value slot; kind "p": two ADJACENT diagonals
# from an interleaved 2*B region (contiguous source, 8-byte descriptors)
# sub 0 (rows of A00|A01|A02), rotation 288k
# sub 1 (A10|A11|0), rotation 2304+288k
# sub 2 (A20|0|A22), rotation 4608+288k
SCATTER = [
    (0, [(0, "s", 0),             # A00 diag: K0
         (N - n, "s", 1),         # A01 k=-n: w*mge
         (N, "s", 2),             # A01 main: -w
         (2 * N - 1, "p", 3)]),   # A02 k=-1 (w*mmod) | A02 main (-w)
    (1, [(0, "s", 5),             # A11 diag
         (2 * N, "s", 6),         # A10 main: rx
         (2 * N + n, "s", 7)]),   # A10 k=+n: rx*mltn
    (2, [(0, "s", 8),             # A22 diag
         (N, "p", 9)]),           # A20 main (ry) | A20 k=+1 (ry*mmodn)
]
NSCATTER = 13

# zero chunks covering sub-band b: rows [288b, 288b+288)
def _cover_end(bnd):
    return -(-(288 * (bnd + 1)) // CHUNK_ROWS)  # ceil


def _build_program() -> bass.Bass:
    nc = bass.Bass()
    f32 = mybir.dt.float32

    pk = nc.declare_dram_parameter("pk", [1, PK], f32, isOutput=False)
    out = nc.declare_dram_parameter("out", [ROWS, M3], f32, isOutput=True)

    with (
        nc.sbuf_tensor([128, ZW], f32) as zt,          # zero tile
        nc.sbuf_tensor([1, PK], f32) as pkb,           # packed inputs
        nc.sbuf_tensor([1, NSLOTS * B], f32) as vv,    # diagonal value vectors
        nc.sbuf_tensor([1, 6 * B], f32) as tmp,        # scratch
        nc.sbuf_tensor([1, 8], f32) as scal,           # scalars
        nc.semaphore("in_sem") as in_sem,
        nc.semaphore("vchain") as vchain,
        nc.semaphore("zsem0") as zsem0,
        nc.semaphore("zsem1") as zsem1,
        nc.semaphore("zsem2") as zsem2,
        nc.semaphore("zsem2a") as zsem2a,
        nc.semaphore("zsem3") as zsem3,
        nc.semaphore("ddma_sem") as ddma_sem,
        nc.Block() as block,
    ):
        # packed-input slices (offsets in elements)
        def pslice(i0, ln):
            return pkb[0:1, i0 : i0 + ln]

        call = pslice(0, N)
        cv = pslice(N, B)
        dkxv = pslice(N + B, B)
        dkyv = pslice(N + 2 * B, B)
        iv = pslice(N + 3 * B, B)
        mge = pslice(N + 4 * B, B)
        mmod = pslice(N + 5 * B, B)
        mltn = pslice(N + 6 * B, B)
        mmodn = pslice(N + 7 * B, B)

        def vslot(s):
            return vv[0:1, s * B : (s + 1) * B]

        def vpair(s, off):
            # stride-2 view over the interleaved pair region at slot s
            return bass.AP(vv, s * B + off, [[NSLOTS * B, 1], [2, B]])

        def tslot(s):
            return tmp[0:1, s * B : (s + 1) * B]

        def sc(i):
            return scal[0:1, i : i + 1]

        mult = mybir.AluOpType.mult
        add = mybir.AluOpType.add

        # chunk -> zero-fill semaphore group: group b must cover all chunks
        # that touch sub-band b's rows and not yet belong to earlier groups
        ZSEMS = [zsem0, zsem1, zsem2, zsem3]
        ZGROUP = [0 if ci < _cover_end(0) else (1 if ci < _cover_end(1) else
                  (2 if ci < NCHUNK - 1 else 3)) for ci in range(NCHUNK)]
        ZGCOUNT = [ZGROUP.count(g) for g in range(4)]
        # the final chunk (108 rows) is split 81+27 so only the last 27
        # rows' scatter descriptors wait for the very end of the fill
        SLIV = ROWS - (NCHUNK - 1) * CHUNK_ROWS       # rows in last chunk
        MAIN2 = B - SLIV                              # sub2 rows before sliver
        W7A = 4374                                    # 81 rows
        S7A = 128 * W7A // M3
        S7B = SLIV - S7A                              # 27 rows

        # zsem increments per group: chunk 0 is issued as four quarter-DMAs
        # so its group gets 64 increments instead of 16
        ZINC = [64 * ZGROUP[:1].count(g) + 16 * ZGROUP[1:-1].count(g)
                for g in range(3)] + [16]

        @block.sync
        def _(sync):
            # zero-fill the whole shard from the (memset) zero tile; chunk 0
            # goes in four quarters chasing the DVE quarter-memsets, so fill
            # data starts ~1.3us after the vector engine boots
            Q = ZW // 4
            g0 = ZSEMS[ZGROUP[0]]
            for qi in range(4):
                sync.wait_ge(vchain, qi + 1)
                dst = bass.AP(out, qi * Q, [[ZW, 128], [1, Q]])
                sync.dma_start(dst, zt[:, qi * Q : (qi + 1) * Q]).then_inc(g0, 16)
            for ci in range(1, NCHUNK - 1):
                dst = bass.AP(out, ci * 128 * ZW, [[ZW, 128], [1, ZW]])
                g = ZGROUP[ci]
                sync.dma_start(dst, zt[:]).then_inc(ZSEMS[g], 16)
            off7 = (NCHUNK - 1) * 128 * ZW
            dst = bass.AP(out, off7, [[W7A, 128], [1, W7A]])
            sync.dma_start(dst, zt[:, :W7A]).then_inc(zsem2a, 16)
            dst = bass.AP(out, off7 + 128 * W7A, [[ZW - W7A, 128], [1, ZW - W7A]])
            sync.dma_start(dst, zt[:, W7A:]).then_inc(zsem3, 16)
            for g, zs in enumerate(ZSEMS):
                sync.wait_ge(zs, ZINC[g])
            sync.wait_ge(zsem2a, 16)

        @block.scalar
        def _(se):
            # input load + diagonal scatter, on the second HWDGE ring so the
            # scatter drains via the SDMA round-robin while the fill runs
            se.dma_start(pkb[:], pk[:]).then_inc(in_sem, 16)
            se.wait_ge(vchain, NVOPS)
            ndma = [0]

            def dodma(bnd, col0, kind, slot0, r0, cnt):
                off = bnd * B * M3 + col0 + r0 * (M3 + 1)
                if kind == "s":
                    dst = bass.AP(out, off, [[M3 + 1, cnt], [1, 1]])
                    src = vv[0:1, slot0 * B + r0 : slot0 * B + r0 + cnt]
                else:  # interleaved adjacent pair
                    dst = bass.AP(out, off, [[M3 + 1, cnt], [1, 2]])
                    src = vv[0:1,
                             slot0 * B + 2 * r0 : slot0 * B + 2 * (r0 + cnt)]
                se.dma_start(dst, src, single_packet=True).then_inc(
                    ddma_sem, 16)
                ndma[0] += 1

            with nc.allow_non_contiguous_dma(reason="diagonal scatter"):
                for bnd, segs in SCATTER:
                    for g in range(bnd + 1):
                        se.wait_ge(ZSEMS[g], ZINC[g])
                    for col0, kind, slot0 in segs:
                        dodma(bnd, col0, kind, slot0, 0,
                              MAIN2 if bnd == 2 else B)
                # sliver: sub-band 2 rows in the split final zero chunk
                se.wait_ge(zsem2a, 16)
                for col0, kind, slot0 in SCATTER[2][1]:
                    dodma(2, col0, kind, slot0, MAIN2, S7A)
                se.wait_ge(ZSEMS[3], ZINC[3])
                for col0, kind, slot0 in SCATTER[2][1]:
                    dodma(2, col0, kind, slot0, MAIN2 + S7A, S7B)
            assert ndma[0] == NSCATTER
            se.wait_ge(ddma_sem, 16 * ndma[0])

        @block.vector
        def _(v):
            # engines have no scoreboarding: serialize the dependent DVE chain
            # through vchain so each op's writeback lands before the next read
            cnt = [0]

            def step(ins):
                cnt[0] += 1
                ins.then_inc(vchain, 1)
                v.wait_ge(vchain, cnt[0])

            Q = ZW // 4
            for qi in range(4):
                cnt[0] += 1
                v.memset(zt[:, qi * Q : (qi + 1) * Q], 0.0).then_inc(vchain, 1)
            v.wait_ge(vchain, 4)
            v.wait_ge(in_sem, 16)
            # dx = 10*sqrt(2)*max(c); invdx = 1/dx
            step(v.reduce_max(sc(0), call, axis=mybir.AxisListType.X))
            step(v.tensor_scalar_mul(sc(1), sc(0), float(DXC)))
            step(v.reciprocal(sc(2), sc(1)))
            # w = mi * rvec ; rv = rvec
            step(v.tensor_scalar(tslot(0), cv, sc(2), float(MI), mult, mult))
            step(v.tensor_scalar_mul(tslot(1), cv, sc(2)))
            step(v.memset(vslot(0), float(K0)))             # A00 diag
            step(v.tensor_mul(vslot(1), tslot(0), mge))     # A01 k=-n
            step(v.tensor_scalar_mul(vslot(2), tslot(0), -1.0))  # A01 main
            step(v.tensor_mul(vpair(3, 0), tslot(0), mmod))      # A02 k=-1
            step(v.tensor_scalar_mul(vpair(3, 1), tslot(0), -1.0))  # A02 main
            # x pass: gx = dkxv*iv; A11 = (1-gx)/(1+gx); A10 = rv/(1+gx)
            step(v.tensor_mul(tslot(2), dkxv, iv))
            step(v.tensor_scalar(tslot(3), tslot(2), 1.0, None, add))
            step(v.reciprocal(tslot(4), tslot(3)))
            step(v.tensor_scalar(tslot(5), tslot(2), -1.0, 1.0, mult, add))
            step(v.tensor_mul(vslot(5), tslot(5), tslot(4)))  # A11 diag
            step(v.tensor_mul(vslot(6), tslot(1), tslot(4)))  # A10 main
            step(v.tensor_mul(vslot(7), vslot(6), mltn))      # A10 k=+n
            # y pass
            step(v.tensor_mul(tslot(2), dkyv, iv))
            step(v.tensor_scalar(tslot(3), tslot(2), 1.0, None, add))
            step(v.reciprocal(tslot(4), tslot(3)))
            step(v.tensor_scalar(tslot(5), tslot(2), -1.0, 1.0, mult, add))
            step(v.tensor_mul(vslot(8), tslot(5), tslot(4)))  # A22 diag
            step(v.tensor_mul(vpair(9, 0), tslot(1), tslot(4)))  # A20 main: ry
            step(v.tensor_mul(vpair(9, 1), vpair(9, 0), mmodn))  # A20 k=+1
            assert cnt[0] == NVOPS, cnt[0]

    return nc


_nc_cache = None


def _get_nc() -> bass.Bass:
    global _nc_cache
    if _nc_cache is None:
        _nc_cache = _build_program()
    return _nc_cache


def _make_in_maps(c, dkx, dky):
    c = np.ascontiguousarray(c, dtype=np.float32)
    cT = np.ascontiguousarray(c.T).reshape(-1)
    dkxT = np.ascontiguousarray(np.asarray(dkx, np.float32).T).reshape(-1)
    dkyT = np.ascontiguousarray(np.asarray(dky, np.float32).T).reshape(-1)
    j = np.arange(N)
    iv = ((j // n) / 2.0).astype(np.float32)
    mge = (j >= n).astype(np.float32)
    mmod = (j % n != 0).astype(np.float32)
    mltn = np.where(j < N - n, -1.0, 0.0).astype(np.float32)
    mmodn = np.where((j + 1) % n != 0, -1.0, 0.0).astype(np.float32)

    in_maps = []
    for k in range(NCORES):
        sl = slice(k * B, (k + 1) * B)
        pk = np.concatenate(
            [c.reshape(-1), cT[sl], dkxT[sl], dkyT[sl], iv[sl],
             mge[sl], mmod[sl], mltn[sl], mmodn[sl]]
        ).astype(np.float32)[None, :]
        assert pk.shape == (1, PK)
        in_maps.append({"pk": pk})
    return in_maps


def _assemble(shards) -> np.ndarray:
    A = np.zeros((M3, M3), dtype=np.float32)
    for k in range(NCORES):
        shard = shards[k]
        for b in range(3):
            g0 = b * N + k * B
            band = shard[b * B : (b + 1) * B]
            if g0:
                A[g0 : g0 + B, g0:] = band[:, : M3 - g0]
                A[g0 : g0 + B, :g0] = band[:, M3 - g0 :]
            else:
                A[:B, :] = band
    return A


def kernel(c, dkx, dky, _trace=False):
    in_maps = _make_in_maps(c, dkx, dky)
    res = run_bass_kernel_spmd(
        _get_nc(), in_maps, core_ids=list(range(NCORES)), trace=_trace
    )
    A = _assemble([res.results[k]["out"] for k in range(NCORES)])
    if _trace:
        return A, res
    return A



# revision 7
# speedup vs baseline: 1.4175x; 1.4175x over previous
"""Trainium2 Bass kernel for nn_EuESN_maml: assemble the 3N x 3N wave-equation
transition matrix A (N = 48*48) from c/dkx/dky fields.

A is all zeros except ~11 diagonals, so the kernel is a DMA memset of the
output plus strided diagonal scatter writes. The run is pure HBM-write-bound
(the fill sustains the ~358 GB/s per-core cap), so the shard is stored as
bfloat16 and widened to f32 on the host while gathering: rounding a value to
bf16 is a <=2^-9 relative error, far inside the 2e-2 gate, and it halves the
bytes the DMAs must push.

Sharding (SPMD, 8 cores): block-row index partitioned. Core k owns rows
[288k, 288k+288) of each of the three N-row block rows of A -> a [864, 6912]
shard per core. Each 288-row sub-band is written column-rotated by its first
global row index so diagonal positions are identical on every core (single
SPMD program); the host un-rotates with two slice copies while gathering.

Engine plan per core:
  vector: memset the zero tile (4 quarters), then the 11 diag value vectors
         (computed in f32 scratch, written back as bf16)
  sync  (HWDGE ring 0): 8 large contiguous ~1.5MB DMAs zero-fill the shard
         at the HBM cap; the first chunk goes in four quarter-DMAs chasing
         the quarter-memsets
  scalar(HWDGE ring 1): strided diagonal scatter DMAs, issued per sub-band as
         soon as the covering zero chunks have landed (overlaps the fill);
         sub-band 2 is split so only the last chunk's rows wait for fill end
"""

import math
import sys

import numpy as np

sys.path.insert(0, "/opt/trn_rl_repo")

import concourse.bass as bass
import concourse.mybir as mybir
from concourse.bass_utils import run_bass_kernel_spmd

# ---- problem constants (hardcoded from the nn_EuESN_maml spec) ----
n = 48
N = n * n            # 2304
M3 = 3 * N           # 6912 (output is M3 x M3)
NCORES = 8
B = N // NCORES      # 288 rows per sub-band
ROWS = 3 * B         # 864 rows per core shard
DT, CN, KP = 1.0, 0.1, 1e-4
MI = 1.0 / (1.0 / DT - KP / 2.0)          # 1/diagM (diagM is constant)
K0 = (1.0 / DT + KP / 2.0) * MI           # A00 diagonal value (constant)
DXC = (DT / CN) * math.sqrt(2.0)          # dx = DXC * max(c)

# zero-fill: NCHUNK contiguous chunks of [128, ZW] elements each
ZW = 5832
NCHUNK = (ROWS * M3) // (128 * ZW)        # 8
CHUNK_ROWS = 128 * ZW // M3               # 108 shard rows per chunk

# packed per-core input: [c.flat (N)] + 8 vectors of length B
PK = N + 8 * B

# number of chained vector-engine ops (vchain semaphore target)
NVOPS = 30

NSLOTS = 11

# scatter DMAs: (sub_band, col0, kind, slot/base)
# kind "s": one diagonal from value slot; kind "p": two ADJACENT diagonals
# from an interleaved 2*B region (contiguous source, 8-byte descriptors)
# sub 0 (rows of A00|A01|A02), rotation 288k
# sub 1 (A10|A11|0), rotation 2304+288k
# sub 2 (A20|0|A22), rotation 4608+288k
SCATTER = [
    (0, [(0, "s", 0),             # A00 diag: K0
         (N - n, "s", 1),         # A01 k=-n: w*mge
         (N, "s", 2),             # A01 main: -w
         (2 * N - 1, "p", 3)]),   # A02 k=-1 (w*mmod) | A02 main (-w)
    (1, [(0, "s", 5),             # A11 diag
         (2 * N, "s", 6),         # A10 main: rx
         (2 * N + n, "s", 7)]),   # A10 k=+n: rx*mltn
    (2, [(0, "s", 8),             # A22 diag
         (N, "p", 9)]),           # A20 main (ry) | A20 k=+1 (ry*mmodn)
]
NSCATTER = 13

# zero chunks covering sub-band b: rows [288b, 288b+288)
def _cover_end(bnd):
    return -(-(288 * (bnd + 1)) // CHUNK_ROWS)  # ceil


def _build_program() -> bass.Bass:
    nc = bass.Bass()
    f32 = mybir.dt.float32
    bf16 = mybir.dt.bfloat16

    pk = nc.declare_dram_parameter("pk", [1, PK], f32, isOutput=False)
    out = nc.declare_dram_parameter("out", [ROWS, M3], bf16, isOutput=True)

    with (
        nc.sbuf_tensor([128, ZW], bf16) as zt,         # zero tile
        nc.sbuf_tensor([1, PK], f32) as pkb,           # packed inputs
        nc.sbuf_tensor([1, NSLOTS * B], bf16) as vv,   # diagonal value vectors
        nc.sbuf_tensor([1, 8 * B], f32) as tmp,        # scratch
        nc.sbuf_tensor([1, 8], f32) as scal,           # scalars
        nc.semaphore("in_sem") as in_sem,
        nc.semaphore("vchain") as vchain,
        nc.semaphore("zsem0") as zsem0,
        nc.semaphore("zsem1") as zsem1,
        nc.semaphore("zsem2") as zsem2,
        nc.semaphore("zsem2a") as zsem2a,
        nc.semaphore("zsem3") as zsem3,
        nc.semaphore("ddma_sem") as ddma_sem,
        nc.Block() as block,
    ):
        # packed-input slices (offsets in elements)
        def pslice(i0, ln):
            return pkb[0:1, i0 : i0 + ln]

        call = pslice(0, N)
        cv = pslice(N, B)
        dkxv = pslice(N + B, B)
        dkyv = pslice(N + 2 * B, B)
        iv = pslice(N + 3 * B, B)
        mge = pslice(N + 4 * B, B)
        mmod = pslice(N + 5 * B, B)
        mltn = pslice(N + 6 * B, B)
        mmodn = pslice(N + 7 * B, B)

        def vslot(s):
            return vv[0:1, s * B : (s + 1) * B]

        def vpair(s, off):
            # stride-2 view over the interleaved pair region at slot s
            return bass.AP(vv, s * B + off, [[NSLOTS * B, 1], [2, B]])

        def tslot(s):
            return tmp[0:1, s * B : (s + 1) * B]

        def sc(i):
            return scal[0:1, i : i + 1]

        mult = mybir.AluOpType.mult
        add = mybir.AluOpType.add

        # chunk -> zero-fill semaphore group: group b must cover all chunks
        # that touch sub-band b's rows and not yet belong to earlier groups
        ZSEMS = [zsem0, zsem1, zsem2, zsem3]
        ZGROUP = [0 if ci < _cover_end(0) else (1 if ci < _cover_end(1) else
                  (2 if ci < NCHUNK - 1 else 3)) for ci in range(NCHUNK)]
        ZGCOUNT = [ZGROUP.count(g) for g in range(4)]
        # the final chunk (108 rows) is split 81+27 so only the last 27
        # rows' scatter descriptors wait for the very end of the fill
        SLIV = ROWS - (NCHUNK - 1) * CHUNK_ROWS       # rows in last chunk
        MAIN2 = B - SLIV                              # sub2 rows before sliver
        W7A = 4374                                    # 81 rows
        S7A = 128 * W7A // M3
        S7B = SLIV - S7A                              # 27 rows

        # zsem increments per group: chunk 0 is issued as four quarter-DMAs
        # so its group gets 64 increments instead of 16
        ZINC = [64 * ZGROUP[:1].count(g) + 16 * ZGROUP[1:-1].count(g)
                for g in range(3)] + [16]

        @block.sync
        def _(sync):
            # zero-fill the whole shard from the (memset) zero tile; chunk 0
            # goes in four quarters chasing the DVE quarter-memsets, so fill
            # data starts ~1.3us after the vector engine boots
            Q = ZW // 4
            g0 = ZSEMS[ZGROUP[0]]
            for qi in range(4):
                sync.wait_ge(vchain, qi + 1)
                dst = bass.AP(out, qi * Q, [[ZW, 128], [1, Q]])
                sync.dma_start(dst, zt[:, qi * Q : (qi + 1) * Q]).then_inc(g0, 16)
            for ci in range(1, NCHUNK - 1):
                dst = bass.AP(out, ci * 128 * ZW, [[ZW, 128], [1, ZW]])
                g = ZGROUP[ci]
                sync.dma_start(dst, zt[:]).then_inc(ZSEMS[g], 16)
            off7 = (NCHUNK - 1) * 128 * ZW
            dst = bass.AP(out, off7, [[W7A, 128], [1, W7A]])
            sync.dma_start(dst, zt[:, :W7A]).then_inc(zsem2a, 16)
            dst = bass.AP(out, off7 + 128 * W7A, [[ZW - W7A, 128], [1, ZW - W7A]])
            sync.dma_start(dst, zt[:, W7A:]).then_inc(zsem3, 16)
            for g, zs in enumerate(ZSEMS):
                sync.wait_ge(zs, ZINC[g])
            sync.wait_ge(zsem2a, 16)

        @block.scalar
        def _(se):
            # input load + diagonal scatter, on the second HWDGE ring so the
            # scatter drains via the SDMA round-robin while the fill runs
            se.dma_start(pkb[:], pk[:]).then_inc(in_sem, 16)
            se.wait_ge(vchain, NVOPS)
            ndma = [0]

            def dodma(bnd, col0, kind, slot0, r0, cnt):
                off = bnd * B * M3 + col0 + r0 * (M3 + 1)
                if kind == "s":
                    dst = bass.AP(out, off, [[M3 + 1, cnt], [1, 1]])
                    src = vv[0:1, slot0 * B + r0 : slot0 * B + r0 + cnt]
                else:  # interleaved adjacent pair
                    dst = bass.AP(out, off, [[M3 + 1, cnt], [1, 2]])
                    src = vv[0:1,
                             slot0 * B + 2 * r0 : slot0 * B + 2 * (r0 + cnt)]
                se.dma_start(dst, src, single_packet=True).then_inc(
                    ddma_sem, 16)
                ndma[0] += 1

            with nc.allow_non_contiguous_dma(reason="diagonal scatter"):
                for bnd, segs in SCATTER:
                    for g in range(bnd + 1):
                        se.wait_ge(ZSEMS[g], ZINC[g])
                    for col0, kind, slot0 in segs:
                        dodma(bnd, col0, kind, slot0, 0,
                              MAIN2 if bnd == 2 else B)
                # sliver: sub-band 2 rows in the split final zero chunk
                se.wait_ge(zsem2a, 16)
                for col0, kind, slot0 in SCATTER[2][1]:
                    dodma(2, col0, kind, slot0, MAIN2, S7A)
                se.wait_ge(ZSEMS[3], ZINC[3])
                for col0, kind, slot0 in SCATTER[2][1]:
                    dodma(2, col0, kind, slot0, MAIN2 + S7A, S7B)
            assert ndma[0] == NSCATTER
            se.wait_ge(ddma_sem, 16 * ndma[0])

        @block.vector
        def _(v):
            # engines have no scoreboarding: serialize the dependent DVE chain
            # through vchain so each op's writeback lands before the next read
            cnt = [0]

            def step(ins):
                cnt[0] += 1
                ins.then_inc(vchain, 1)
                v.wait_ge(vchain, cnt[0])

            Q = ZW // 4
            for qi in range(4):
                cnt[0] += 1
                v.memset(zt[:, qi * Q : (qi + 1) * Q], 0.0).then_inc(vchain, 1)
            v.wait_ge(vchain, 4)
            v.wait_ge(in_sem, 16)
            # dx = 10*sqrt(2)*max(c); invdx = 1/dx
            step(v.reduce_max(sc(0), call, axis=mybir.AxisListType.X))
            step(v.tensor_scalar_mul(sc(1), sc(0), float(DXC)))
            step(v.reciprocal(sc(2), sc(1)))
            # w = mi * rvec ; rv = rvec
            step(v.tensor_scalar(tslot(0), cv, sc(2), float(MI), mult, mult))
            step(v.tensor_scalar_mul(tslot(1), cv, sc(2)))
            step(v.memset(vslot(0), float(K0)))             # A00 diag
            step(v.tensor_mul(vslot(1), tslot(0), mge))     # A01 k=-n
            step(v.tensor_scalar_mul(vslot(2), tslot(0), -1.0))  # A01 main
            step(v.tensor_mul(vpair(3, 0), tslot(0), mmod))      # A02 k=-1
            step(v.tensor_scalar_mul(vpair(3, 1), tslot(0), -1.0))  # A02 main
            # x pass: gx = dkxv*iv; A11 = (1-gx)/(1+gx); A10 = rv/(1+gx)
            step(v.tensor_mul(tslot(2), dkxv, iv))
            step(v.tensor_scalar(tslot(3), tslot(2), 1.0, None, add))
            step(v.reciprocal(tslot(4), tslot(3)))
            step(v.tensor_scalar(tslot(5), tslot(2), -1.0, 1.0, mult, add))
            step(v.tensor_mul(vslot(5), tslot(5), tslot(4)))  # A11 diag
            step(v.tensor_mul(tslot(6), tslot(1), tslot(4)))  # rx (f32)
            step(v.tensor_copy(vslot(6), tslot(6)))           # A10 main
            step(v.tensor_mul(vslot(7), tslot(6), mltn))      # A10 k=+n
            # y pass
            step(v.tensor_mul(tslot(2), dkyv, iv))
            step(v.tensor_scalar(tslot(3), tslot(2), 1.0, None, add))
            step(v.reciprocal(tslot(4), tslot(3)))
            step(v.tensor_scalar(tslot(5), tslot(2), -1.0, 1.0, mult, add))
            step(v.tensor_mul(vslot(8), tslot(5), tslot(4)))  # A22 diag
            step(v.tensor_mul(tslot(6), tslot(1), tslot(4)))  # ry (f32)
            step(v.tensor_copy(vpair(9, 0), tslot(6)))        # A20 main
            step(v.tensor_mul(vpair(9, 1), tslot(6), mmodn))  # A20 k=+1
            assert cnt[0] == NVOPS, cnt[0]

    return nc


_nc_cache = None


def _get_nc() -> bass.Bass:
    global _nc_cache
    if _nc_cache is None:
        _nc_cache = _build_program()
    return _nc_cache


def _make_in_maps(c, dkx, dky):
    c = np.ascontiguousarray(c, dtype=np.float32)
    cT = np.ascontiguousarray(c.T).reshape(-1)
    dkxT = np.ascontiguousarray(np.asarray(dkx, np.float32).T).reshape(-1)
    dkyT = np.ascontiguousarray(np.asarray(dky, np.float32).T).reshape(-1)
    j = np.arange(N)
    iv = ((j // n) / 2.0).astype(np.float32)
    mge = (j >= n).astype(np.float32)
    mmod = (j % n != 0).astype(np.float32)
    mltn = np.where(j < N - n, -1.0, 0.0).astype(np.float32)
    mmodn = np.where((j + 1) % n != 0, -1.0, 0.0).astype(np.float32)

    in_maps = []
    for k in range(NCORES):
        sl = slice(k * B, (k + 1) * B)
        pk = np.concatenate(
            [c.reshape(-1), cT[sl], dkxT[sl], dkyT[sl], iv[sl],
             mge[sl], mmod[sl], mltn[sl], mmodn[sl]]
        ).astype(np.float32)[None, :]
        assert pk.shape == (1, PK)
        in_maps.append({"pk": pk})
    return in_maps


def _assemble(shards) -> np.ndarray:
    A = np.zeros((M3, M3), dtype=np.float32)
    for k in range(NCORES):
        # device shard is bf16; widening to f32 is exact
        shard = np.asarray(shards[k]).astype(np.float32)
        for b in range(3):
            g0 = b * N + k * B
            band = shard[b * B : (b + 1) * B]
            if g0:
                A[g0 : g0 + B, g0:] = band[:, : M3 - g0]
                A[g0 : g0 + B, :g0] = band[:, M3 - g0 :]
            else:
                A[:B, :] = band
    return A


def kernel(c, dkx, dky, _trace=False):
    in_maps = _make_in_maps(c, dkx, dky)
    res = run_bass_kernel_spmd(
        _get_nc(), in_maps, core_ids=list(range(NCORES)), trace=_trace
    )
    A = _assemble([res.results[k]["out"] for k in range(NCORES)])
    if _trace:
        return A, res
    return A



# revision 15
# speedup vs baseline: 1.6546x; 1.1672x over previous
"""Trainium2 Bass kernel for nn_EuESN_maml: assemble the 3N x 3N wave-equation
transition matrix A (N = 48*48) from c/dkx/dky fields.

A is all zeros except ~11 diagonals, so the kernel is a DMA memset of the
output plus strided diagonal scatter writes. The run is pure HBM-write-bound
(the fill sustains the ~358 GB/s per-core cap), so the shard is stored as
bfloat16 and widened to f32 on the host while gathering: rounding a value to
bf16 is a <=2^-9 relative error, far inside the 2e-2 gate, and it halves the
bytes the DMAs must push.

Sharding (SPMD, 8 cores): block-row index partitioned. Core k owns rows
[288k, 288k+288) of each of the three N-row block rows of A -> a [864, 6912]
shard per core. Each 288-row sub-band is written column-rotated by its first
global row index so diagonal positions are identical on every core (single
SPMD program); the host un-rotates with two slice copies while gathering.

Engine plan per core:
  vector: half the zero-tile memset, then the diagonal value vectors,
         computed on 96 partitions ([96,3] row-major packing of each
         288-vector) with the x/y chains interleaved to hide the
         no-scoreboard semaphore latency; the global max(c) reduction
         finishes through a [128,128] DVE transpose
  gpsimd: the other zero-tile half, then the constructed-tile memset
  sync  (HWDGE ring 0): 12 contiguous zero-fill DMAs (64 shard rows each)
         for rows 0..737, then the constructed tile (rows 738..863) whose
         SBUF image already contains its three diagonals, so nothing has
         to scatter after the fill ends
  scalar(HWDGE ring 1): input load, the inv-dx broadcast, the three
         constructed-tile strip writes (SBUF->SBUF diagonal APs), and the
         11 single-diagonal scatter DMAs, each issued as soon as the zero
         chunks covering its rows have landed (fully inside the fill)
"""

import math
import sys

import numpy as np

sys.path.insert(0, "/opt/trn_rl_repo")

import concourse.bass as bass
import concourse.mybir as mybir
from concourse.bass_utils import run_bass_kernel_spmd

# ---- problem constants (hardcoded from the nn_EuESN_maml spec) ----
n = 48
N = n * n            # 2304
M3 = 3 * N           # 6912 (output is M3 x M3)
NCORES = 8
B = N // NCORES      # 288 rows per sub-band
ROWS = 3 * B         # 864 rows per core shard
DT, CN, KP = 1.0, 0.1, 1e-4
MI = 1.0 / (1.0 / DT - KP / 2.0)          # 1/diagM (diagM is constant)
K0 = (1.0 / DT + KP / 2.0) * MI           # A00 diagonal value (constant)
DXC = (DT / CN) * math.sqrt(2.0)          # dx = DXC * max(c)

# zero tile [128, ZTW] bf16; each full fill DMA covers 64 shard rows
ZTW = 3456
CTR = 126            # constructed-tile rows: shard rows 738..863
ZROWS = ROWS - CTR   # 738 zero-filled rows
NFULL = ZROWS // 64  # 11 full 64-row DMAs
PROWS = ZROWS - NFULL * 64                # 34-row partial piece
J0 = B - CTR         # first sub-band-2-local row handled by the tile (162)

# value slots, [96, 3] row-major each (value r lives at [r//3, slot*3+r%3])
NSLOT = 10
VW = 3 * NSLOT
# sub-band scatter: (sub_band, count, [(col0, slot), ...])
SCATTER = [
    (0, B, [(0, 0),          # A00 diag: K0
            (N - n, 1),      # A01 k=-n: w*mge
            (N, 2),          # A01 main: -w
            (2 * N - 1, 3),  # A02 k=-1: w*mmod
            (2 * N, 2)]),    # A02 main: -w (same vector as A01 main)
    (1, B, [(0, 4),          # A11 diag
            (2 * N, 5),      # A10 main: rx
            (2 * N + n, 6)]),  # A10 k=+n: rx*mltn
    (2, J0, [(0, 7),         # A22 diag
             (N, 8),         # A20 main: ry
             (N + 1, 9)]),   # A20 k=+1: ry*mmodn
]
# constructed-tile strips: (tile col0, slot) for rows j = J0..287
STRIPS = [(J0, 7), (N + J0, 8), (N + J0 + 1, 9)]

NVOPS = 30
# packed input: 8 value vectors as [96, 3] at cols 0..23, then the c grid
# as [32, 72] at cols 24..95 (32 partitions so one 32x32 block transpose
# finishes the global max)
VCOL = 0
CCOL = 24
PKW = CCOL + 72


def _build_program() -> bass.Bass:
    nc = bass.Bass()
    f32 = mybir.dt.float32
    bf16 = mybir.dt.bfloat16

    pk = nc.declare_dram_parameter("pk", [128, PKW], f32, isOutput=False)
    out = nc.declare_dram_parameter("out", [ROWS, M3], bf16, isOutput=True)

    from contextlib import ExitStack

    with ExitStack() as ctx:
        ec = ctx.enter_context
        zt = ec(nc.sbuf_tensor([128, ZTW], bf16))      # zero tile
        ct = ec(nc.sbuf_tensor([128, M3], bf16))       # constructed tile
        pkb = ec(nc.sbuf_tensor([128, PKW], f32))      # packed inputs
        vv = ec(nc.sbuf_tensor([96, VW], bf16))        # diag value slots
        tmp = ec(nc.sbuf_tensor([96, 36], f32))        # [96,3] scratch x12
        tsq = ec(nc.sbuf_tensor([32, 32], f32))        # transpose in
        tsqT = ec(nc.sbuf_tensor([32, 32], f32))       # transpose out
        rowA = ec(nc.sbuf_tensor([1, 96], f32))        # inv-dx replicated
        ib = ec(nc.sbuf_tensor([96, 1], f32))          # inv-dx per partition
        scal = ec(nc.sbuf_tensor([1, 8], f32))         # scalars
        in_sem = ec(nc.semaphore("in_sem"))
        vchain = ec(nc.semaphore("vchain"))
        gz = ec(nc.semaphore("gz"))
        ctm = ec(nc.semaphore("ctm"))
        bsem = ec(nc.semaphore("bsem"))
        zsemA = ec(nc.semaphore("zsemA"))
        zsemB = ec(nc.semaphore("zsemB"))
        zsemC = ec(nc.semaphore("zsemC"))
        strip_sem = ec(nc.semaphore("strip_sem"))
        ddma = ec(nc.semaphore("ddma"))
        ctd = ec(nc.semaphore("ctd"))
        block = ec(nc.Block())
        def vslot(s, p0=0, np_=96):
            # [np_, 3] view of value slot s starting at partition p0
            return vv[p0 : p0 + np_, s * 3 : s * 3 + 3]

        def tslot(s):
            return tmp[0:96, s * 3 : (s + 1) * 3]

        def sc(i):
            return scal[0:1, i : i + 1]

        def pvec(i):
            return pkb[0:96, VCOL + 3 * i : VCOL + 3 * i + 3]

        cv2, dkx2, dky2 = pvec(0), pvec(1), pvec(2)
        iv2, mge2, mmod2 = pvec(3), pvec(4), pvec(5)
        mltn2, mmodn2 = pvec(6), pvec(7)

        mult = mybir.AluOpType.mult
        add = mybir.AluOpType.add

        @block.sync
        def _(sync):
            # zero-fill rows 0..737 as 64-row pieces + one 34-row piece;
            # each full DMA re-reads the whole [128, ZTW] zero tile. The
            # first piece goes as two 32-row halves that only read the
            # vector-memset half of the tile, so the fill starts without
            # waiting for the gpsimd half
            sync.wait_ge(vchain, 1)
            for q in range(2):
                dst = bass.AP(out, q * 32 * M3,
                              [[ZTW // 2, 128], [1, ZTW // 2]])
                sync.dma_start(dst, zt[:, : ZTW // 2]).then_inc(zsemA, 16)
            sync.wait_ge(gz, 1)
            for h in range(1, NFULL):
                dst = bass.AP(out, h * 64 * M3, [[ZTW, 128], [1, ZTW]])
                zs = zsemA if h < 5 else (zsemB if h < 9 else zsemC)
                sync.dma_start(dst, zt[:]).then_inc(zs, 16)
            dst = bass.AP(out, NFULL * 64 * M3,
                          [[ZTW, 2 * PROWS], [1, ZTW]])
            sync.dma_start(dst, zt[0 : 2 * PROWS, :]).then_inc(zsemC, 16)
            # constructed tile last: rows 738..863 with diagonals baked in
            sync.wait_ge(strip_sem, 48)
            dst = bass.AP(out, ZROWS * M3, [[M3, CTR], [1, M3]])
            sync.dma_start(dst, ct[0:CTR, :]).then_inc(ctd, 16)
            sync.wait_ge(ctd, 16)

        @block.gpsimd
        def _(g):
            g.memset(zt[:, ZTW // 2 :], 0.0).then_inc(gz, 1)
            g.memset(ct[:], 0.0).then_inc(ctm, 1)

        @block.scalar
        def _(se):
            se.dma_start(pkb[:], pk[:]).then_inc(in_sem, 16)
            # inv-dx broadcast [1,96] -> [96,1] once the DVE computed it
            se.wait_ge(vchain, 11)
            with nc.allow_non_contiguous_dma(reason="partition broadcast"):
                se.dma_start(ib[0:96, 0:1], rowA[0:1, :]).then_inc(bsem, 16)
            ndma = [0]

            def diag(base, col0, cnt, src):
                dst = bass.AP(out, base + col0, [[M3 + 1, cnt], [1, 1]])
                se.dma_start(dst, src, single_packet=True).then_inc(ddma, 16)
                ndma[0] += 1

            with nc.allow_non_contiguous_dma(reason="diagonal scatter"):
                # constructed-tile strips (SBUF->SBUF) as soon as the tile
                # is memset and the value slots are ready
                se.wait_ge(vchain, NVOPS)
                se.wait_ge(ctm, 1)
                for col0, s in STRIPS:
                    dst = bass.AP(ct, col0, [[M3 + 1, CTR], [1, 1]])
                    se.dma_start(dst, vslot(s, J0 // 3, CTR // 3),
                                 single_packet=True).then_inc(strip_sem, 16)
                # diagonal scatters, gated per sub-band on the zero fill
                for (bnd, cnt, segs), zs, tgt in zip(
                        SCATTER, (zsemA, zsemB, zsemC), (96, 64, 48)):
                    se.wait_ge(zs, tgt)
                    for col0, s in segs:
                        diag(bnd * B * M3, col0, cnt, vslot(s, 0, cnt // 3))
            assert ndma[0] == 11
            se.wait_ge(ddma, 16 * ndma[0])

        @block.vector
        def _(v):
            # no scoreboarding: dependent ops are serialized through vchain,
            # with producers/consumers interleaved >=2 apart so the
            # semaphore round-trip hides behind the interposed op
            cnt = [0]

            def op(ins, wait=None):
                cnt[0] += 1
                ins.then_inc(vchain, 1)
                if wait is not None:
                    v.wait_ge(vchain, wait)

            op(v.memset(zt[:, : ZTW // 2], 0.0))           # 1 zero-tile half
            op(v.memset(vslot(0), float(K0)))              # 2 A00 diag const
            op(v.memset(rowA[:], 1.0))                     # 3
            v.wait_ge(in_sem, 16)
            # global max(c): [32,72] reduce -> 32x32 block transpose puts
            # the 32 partials into row 0 -> [1,32] reduce
            op(v.reduce_max(tsq[0:32, 0:1], pkb[0:32, CCOL:PKW],
                            axis=mybir.AxisListType.X), wait=4)
            op(v.transpose(tsqT[:], tsq[:]), wait=5)
            op(v.reduce_max(sc(0), tsqT[0:1, 0:32],
                            axis=mybir.AxisListType.X), wait=6)
            op(v.tensor_scalar_mul(sc(1), sc(0), float(DXC)), wait=7)
            op(v.reciprocal(sc(2), sc(1)), wait=8)
            op(v.tensor_mul(tslot(0), dkx2, iv2))          # 9  gx
            op(v.tensor_mul(tslot(4), dky2, iv2))          # 10 gy
            op(v.tensor_scalar_mul(rowA[:], rowA[:], sc(2)), wait=9)  # 11
            op(v.tensor_scalar(tslot(1), tslot(0), 1.0, None, add), wait=10)
            op(v.tensor_scalar(tslot(5), tslot(4), 1.0, None, add), wait=12)
            op(v.reciprocal(tslot(2), tslot(1)), wait=13)  # 14 rxi
            op(v.reciprocal(tslot(6), tslot(5)))           # 15 ryi
            op(v.tensor_scalar(tslot(3), tslot(0), -1.0, 1.0, mult, add))
            op(v.tensor_scalar(tslot(7), tslot(4), -1.0, 1.0, mult, add),
               wait=16)
            op(v.tensor_mul(vslot(4), tslot(3), tslot(2)), wait=17)  # A11
            op(v.tensor_mul(vslot(7), tslot(7), tslot(6)))           # A22
            v.wait_ge(bsem, 16)
            op(v.tensor_scalar(tslot(9), cv2, ib[:, 0:1], float(MI),
                               mult, mult))                # 20 w
            op(v.tensor_scalar_mul(tslot(8), cv2, ib[:, 0:1]), wait=20)  # rv
            op(v.tensor_mul(vslot(1), tslot(9), mge2), wait=21)
            op(v.tensor_mul(tslot(10), tslot(8), tslot(2)))          # 23 rx
            op(v.tensor_scalar_mul(vslot(2), tslot(9), -1.0))
            op(v.tensor_mul(tslot(11), tslot(8), tslot(6)))          # 25 ry
            op(v.tensor_mul(vslot(3), tslot(9), mmod2), wait=23)
            op(v.tensor_copy(vslot(5), tslot(10)))                   # A10 m
            op(v.tensor_mul(vslot(6), tslot(10), mltn2), wait=25)
            op(v.tensor_copy(vslot(8), tslot(11)))                   # A20 m
            op(v.tensor_mul(vslot(9), tslot(11), mmodn2))
            assert cnt[0] == NVOPS, cnt[0]

    return nc


_nc_cache = None


def _get_nc() -> bass.Bass:
    global _nc_cache
    if _nc_cache is None:
        _nc_cache = _build_program()
    return _nc_cache


def _make_in_maps(c, dkx, dky):
    c = np.ascontiguousarray(c, dtype=np.float32)
    cT = np.ascontiguousarray(c.T).reshape(-1)
    dkxT = np.ascontiguousarray(np.asarray(dkx, np.float32).T).reshape(-1)
    dkyT = np.ascontiguousarray(np.asarray(dky, np.float32).T).reshape(-1)
    j = np.arange(N)
    iv = ((j // n) / 2.0).astype(np.float32)
    mge = (j >= n).astype(np.float32)
    mmod = (j % n != 0).astype(np.float32)
    mltn = np.where(j < N - n, -1.0, 0.0).astype(np.float32)
    mmodn = np.where((j + 1) % n != 0, -1.0, 0.0).astype(np.float32)

    in_maps = []
    for k in range(NCORES):
        sl = slice(k * B, (k + 1) * B)
        pk = np.zeros((128, PKW), dtype=np.float32)
        pk[0:32, CCOL:PKW] = c.reshape(32, 72)
        for i, vec in enumerate(
                [cT, dkxT, dkyT, iv, mge, mmod, mltn, mmodn]):
            pk[0:96, VCOL + 3 * i : VCOL + 3 * i + 3] = vec[sl].reshape(96, 3)
        in_maps.append({"pk": pk})
    return in_maps


def _assemble(shards) -> np.ndarray:
    A = np.zeros((M3, M3), dtype=np.float32)
    for k in range(NCORES):
        # device shard is bf16; widening to f32 is exact
        shard = np.asarray(shards[k]).astype(np.float32)
        for b in range(3):
            g0 = b * N + k * B
            band = shard[b * B : (b + 1) * B]
            if g0:
                A[g0 : g0 + B, g0:] = band[:, : M3 - g0]
                A[g0 : g0 + B, :g0] = band[:, M3 - g0 :]
            else:
                A[:B, :] = band
    return A


def kernel(c, dkx, dky, _trace=False):
    in_maps = _make_in_maps(c, dkx, dky)
    res = run_bass_kernel_spmd(
        _get_nc(), in_maps, core_ids=list(range(NCORES)), trace=_trace
    )
    A = _assemble([res.results[k]["out"] for k in range(NCORES)])
    if _trace:
        return A, res
    return A
